# revision 48
# baseline (speedup 1.0000x reference)
"""Trainium2 Bass kernel for the GCNet-style 3D attention module.

Math: the module's final output collapses to

    out[b, c, n] = S[b, c] + relu(wres @ x)[b, c, n]

because `parallel = softmax(ca + sa)` over n is independent of the per-channel
offset sa[c], so softmax(ca+sa) == softmax(ca) and every recombination term
reduces to per-(b,c) scalars:

    S = a * sa * sum_n(ca) + (1 - a)

with a, sa derived from tiny per-sample reductions over n = D*H*W:
  - q = relu(wqr@x)  -> Z = sum exp(q), ctx = sum relu(wvr@x) * exp(q)
  - gsum = sum relu(wql@x)  -> avg_x = softmax(gsum/n)
  - sca = sum sigmoid(avg_x . relu(wvl@x))

Device work = 3 streaming passes over x, sharded (batch, n/2) over 8 cores:
  K1: per-shard partials (ctx numerator, Z, gsum). Conv outputs are computed
      in [position, channel] orientation (x chunks as the stationary matmul
      operand) so exp runs at full 128-lane occupancy and the n-reductions
      become tiny PSUM matmuls against an fp16 [v|g|ones] staging tile.
  K2: per-shard sca partial (needs avg_x from K1); same orientation, the
      avg_x dot is a DVE multiply + free-axis reduce, sigmoid batched 32-wide.
  K3: out = S + relu(wres @ x)   (needs S from K2); single fused DVE op per
      tile, loads and stores on separate HWDGE queues (bandwidth-bound).
K1/K2 alternate their input loads between the SP HWDGE queue and the GpSimd
SWDGE queue: a single queue serializes on per-DMA sequencer occupancy (~3us
per 512KB load) and was the dominant bottleneck, not bandwidth.
Host merges the tiny partials between passes in float64.
All matmuls stream float32r (full-rate fp32; moving operands < 256 wide pay
4 cyc/row but casting x to bf16/fp16 costs more precision than it is worth);
small-operand staging uses fp16 (the output is dominated by the per-channel
scalar S, so conv-path rounding is ~1e-6 of the result).
"""

import numpy as np

import concourse.bacc as bacc
import concourse.tile as tile
from concourse import mybir
from concourse.bass_utils import run_bass_kernel_spmd

B, C, D, H, W = 4, 64, 32, 64, 64
CH = C // 2
N = D * H * W          # 131072 positions per sample
NCORES = 8
NS = N // 2            # 65536 positions per core shard
T = 512                # matmul tile (positions)
XT = 2048              # DMA staging width (positions)
NLOAD = NS // XT       # 32 staged loads
INNER = XT // T        # 4 matmul tiles per load
NT = NS // T           # 128 matmul tiles total

F32 = mybir.dt.float32
F32R = mybir.dt.float32r
BF16 = mybir.dt.bfloat16
FP16 = mybir.dt.float16
AF = mybir.ActivationFunctionType

_cache = {}


def _build_k1():
    """Pass A: for each (b, half) shard compute part[65, 16]:
    col 2k = [sum v*e | junk | Z]_chunk_k, col 2k+1 = [junk | gsum | n]_k.
    Orientation-2: conv outputs computed as [pos, ch] chunks (x as stationary
    operand), so exp/reductions run at full lane occupancy with no transposes.
    1024-position tiles (8 chunks) halve the number of cross-engine handoffs."""
    nc = bacc.Bacc("TRN2", target_bir_lowering=False)
    x = nc.declare_dram_parameter("x", [C, NS], F32R, isOutput=False)
    w = nc.declare_dram_parameter("w", [C, 66], F32R, isOutput=False)
    part = nc.declare_dram_parameter("part", [65, 16], F32, isOutput=True)
    CHN = 8             # 128-position chunks per tile
    T1 = 128 * CHN      # 1024 positions per tile
    NT1 = NS // T1      # 64 tiles
    IN1 = XT // T1      # 2 tiles per staged load
    S1 = 4              # chunks < S1 relu'd on ACT, rest on DVE
    SKEW = 2            # ctx matmuls issued SKEW tiles behind the conv matmuls
    NROT = 4

    with tile.TileContext(nc) as tc:
        with tc.tile_pool(name="stage", bufs=5) as stage, \
             tc.tile_pool(name="work", bufs=4) as work, \
             tc.tile_pool(name="vq", bufs=1) as vqp, \
             tc.tile_pool(name="acc", bufs=1) as acc, \
             tc.tile_pool(name="ps", bufs=3, space="PSUM") as ps, \
             tc.tile_pool(name="psx", bufs=1, space="PSUM") as psx, \
             tc.tile_pool(name="psc", bufs=1, space="PSUM") as psc:
            wt = acc.tile([C, 66], F32R)
            nc.sync.dma_start(out=wt, in_=w[:, :])
            # warm matmul: absorb the weight-DMA dependency once
            pwarm = psc.tile([66, 4], F32)
            nc.tensor.matmul(pwarm, lhsT=wt, rhs=wt[:, 0:4], start=True, stop=True)

            # persistent rotation tiles: [128, CHN, 66] fp16, col 64 == 1.0
            vqs = []
            for r in range(NROT):
                vq_t = vqp.tile([128, CHN, 66], FP16, tag=f"vq{r}")
                nc.vector.memset(vq_t, 1.0)
                vqs.append(vq_t)
            es = []
            for r in range(NROT):
                e_t = vqp.tile([128, 2 * CHN], FP16, tag=f"e{r}")
                nc.vector.memset(e_t, 1.0)
                es.append(e_t)

            acc_sb = acc.tile([65, 2 * CHN], F32)
            nc.vector.memset(acc_sb, 0.0)

            def ctx_mms(t):
                pv, pe_ = vqs[t % NROT], es[t % NROT]
                ctx_ps = psx.tile([65, 2 * CHN], F32, tag="ctxp")
                for k in range(CHN):
                    nc.tensor.matmul(ctx_ps[:, 2 * k:2 * k + 2],
                                     lhsT=pv[:, k, 0:65],
                                     rhs=pe_[:, 2 * k:2 * k + 2],
                                     start=True, stop=True)
                nc.vector.tensor_add(out=acc_sb, in0=acc_sb, in1=ctx_ps)

            for ld in range(NLOAD):
                xs = stage.tile([C, XT], F32R, tag="xs")
                dma_eng = nc.sync if ld % 2 == 0 else nc.gpsimd
                dma_eng.dma_start(out=xs, in_=x[:, ld * XT:(ld + 1) * XT])
                for kk in range(IN1):
                    t = ld * IN1 + kk
                    vq = vqs[t % NROT]
                    e_sb = es[t % NROT]
                    # conv chunks: pa[:, k, c] = channel c at position p
                    # (w cols: 0 = wqr, 1:33 = wvr, 33:65 = wql)
                    pa = ps.tile([128, CHN, 66], F32, tag="pa")
                    for k in range(CHN):
                        nc.tensor.matmul(
                            pa[:, k, 0:66],
                            lhsT=xs[:, (kk * CHN + k) * 128:(kk * CHN + k + 1) * 128],
                            rhs=wt, start=True, stop=True)
                    # relu of v|g channels straight into the lhsT staging tile
                    # (ones col 64 stays intact)
                    nc.scalar.activation(
                        out=vq[:, 0:S1, 0:64], in_=pa[:, 0:S1, 1:65],
                        func=AF.Relu)
                    nc.vector.tensor_scalar_max(
                        out=vq[:, S1:CHN, 0:64], in0=pa[:, S1:CHN, 1:65],
                        scalar1=0.0)
                    # e = exp(relu(q)) = max(exp(q), 1)
                    escr = work.tile([128, CHN], FP16, tag="escr")
                    nc.scalar.activation(out=escr, in_=pa[:, :, 0], func=AF.Exp)
                    nc.vector.tensor_scalar_max(out=e_sb[:, 0:2 * CHN:2], in0=escr,
                                                scalar1=1.0)
                    # ctx/gsum/Z reduction matmuls, SKEW tiles behind (avoids
                    # PE ping-pong); per-tile psum groups + DVE accumulate
                    # (cross-tile PSUM accumulation groups get reordered by
                    # the scheduler and drop tiles)
                    if t >= SKEW:
                        ctx_mms(t - SKEW)

            for t in range(NT1 - SKEW, NT1):
                ctx_mms(t)

            nc.sync.dma_start(out=part[:, :], in_=acc_sb)
    nc.finalize()
    return nc


def _build_k2():
    """Pass B: per-partition partial sums of sigmoid(avg_x . relu(wvl@x)).
    Orientation-2: theta computed as [pos, ch] chunks so the avg_x dot is a
    DVE multiply + free-axis reduce (no per-chunk weight loads on PE)."""
    nc = bacc.Bacc("TRN2", target_bir_lowering=False)
    x = nc.declare_dram_parameter("x", [C, NS], F32R, isOutput=False)
    w = nc.declare_dram_parameter("w", [C, CH], F32R, isOutput=False)
    avgx = nc.declare_dram_parameter("avgx", [1, CH], F32, isOutput=False)
    part = nc.declare_dram_parameter("part", [128, 16], F32, isOutput=True)
    S2 = 4  # chunks 0..S2-1 relu'd on ACT, rest on DVE
    GRP = 8

    with tile.TileContext(nc) as tc:
        with tc.tile_pool(name="stage", bufs=4) as stage, \
             tc.tile_pool(name="work", bufs=6) as work, \
             tc.tile_pool(name="small", bufs=2) as small, \
             tc.tile_pool(name="acc", bufs=1) as acc, \
             tc.tile_pool(name="ps", bufs=6, space="PSUM") as ps, \
             tc.tile_pool(name="psw", bufs=1, space="PSUM") as psw:
            wt = acc.tile([C, CH], F32R)
            nc.sync.dma_start(out=wt, in_=w[:, :])
            av = acc.tile([1, CH], F32)
            nc.sync.dma_start(out=av, in_=avgx[:, :])
            # avg_x broadcast to all 128 partitions, replicated x4 along free
            av_b = acc.tile([128, 4, CH], FP16)
            av16 = acc.tile([1, CH], FP16)
            nc.vector.tensor_copy(out=av16, in_=av)
            nc.gpsimd.partition_broadcast(av_b[:, 0, :], av16[0:1, :])
            nc.vector.tensor_copy(out=av_b[:, 1, :], in_=av_b[:, 0, :])
            nc.vector.tensor_copy(out=av_b[:, 2:4, :].rearrange("p a b -> p (a b)"),
                                  in_=av_b[:, 0:2, :].rearrange("p a b -> p (a b)"))
            pwarm = psw.tile([CH, 4], F32, tag="warm")
            nc.tensor.matmul(pwarm, lhsT=wt, rhs=wt[:, 0:4], start=True, stop=True)

            sca_cols = acc.tile([128, NT // GRP], F32)
            gather = None

            for ld in range(NLOAD):
                xs = stage.tile([C, XT], F32R, tag="xs")
                dma_eng = nc.sync if ld % 2 == 0 else nc.gpsimd
                dma_eng.dma_start(out=xs, in_=x[:, ld * XT:(ld + 1) * XT])
                for kk in range(INNER):
                    t = ld * INNER + kk
                    r = t % GRP
                    if r == 0:
                        gather = work.tile([128, 4 * GRP], F32, tag="gather")
                    pa = ps.tile([128, 4, CH], F32, tag="pa")
                    for k in range(4):
                        nc.tensor.matmul(
                            pa[:, k, :],
                            lhsT=xs[:, (kk * 4 + k) * 128:(kk * 4 + k + 1) * 128],
                            rhs=wt, start=True, stop=True)
                    th = work.tile([128, 4, CH], FP16, tag="th")
                    if S2 > 0:
                        nc.scalar.activation(
                            out=th[:, 0:S2, :].rearrange("p a b -> p (a b)"),
                            in_=pa[:, 0:S2, :].rearrange("p a b -> p (a b)"),
                            func=AF.Relu)
                    if S2 < 4:
                        nc.vector.tensor_scalar_max(
                            out=th[:, S2:4, :].rearrange("p a b -> p (a b)"),
                            in0=pa[:, S2:4, :].rearrange("p a b -> p (a b)"),
                            scalar1=0.0)
                    prod = work.tile([128, 4, CH], FP16, tag="prod")
                    nc.vector.tensor_mul(out=prod, in0=th, in1=av_b)
                    nc.vector.reduce_sum(out=gather[:, 4 * r:4 * r + 4], in_=prod,
                                         axis=mybir.AxisListType.X)
                    if r == GRP - 1:
                        g = t // GRP
                        scr = work.tile([128, 4 * GRP], BF16, tag="scr")
                        nc.scalar.activation(out=scr, in_=gather, func=AF.Sigmoid,
                                             accum_out=sca_cols[:, g:g + 1])

            out_sb = small.tile([128, NT // GRP], F32)
            nc.vector.tensor_copy(out=out_sb, in_=sca_cols)
            nc.sync.dma_start(out=part[:, :], in_=out_sb)
    nc.finalize()
    return nc


def _build_k3():
    """Pass C: y = S + relu(wres @ x)."""
    nc = bacc.Bacc("TRN2", target_bir_lowering=False)
    x = nc.declare_dram_parameter("x", [C, NS], F32R, isOutput=False)
    w = nc.declare_dram_parameter("w", [C, C], F32R, isOutput=False)
    svec = nc.declare_dram_parameter("svec", [C, 1], F32, isOutput=False)
    y = nc.declare_dram_parameter("y", [C, NS], F32, isOutput=True)

    with tile.TileContext(nc) as tc:
        with tc.tile_pool(name="stage", bufs=4) as stage, \
             tc.tile_pool(name="ostage", bufs=4) as ostage, \
             tc.tile_pool(name="work", bufs=8) as work, \
             tc.tile_pool(name="acc", bufs=1) as acc, \
             tc.tile_pool(name="psw", bufs=1, space="PSUM") as psw, \
             tc.tile_pool(name="ps", bufs=6, space="PSUM") as ps:
            wt = acc.tile([C, C], F32R)
            nc.sync.dma_start(out=wt, in_=w[:, :])
            sv = acc.tile([C, 1], F32)
            nc.sync.dma_start(out=sv, in_=svec[:, :])
            sb_ = acc.tile([C, T], F32)
            nc.vector.tensor_copy(out=sb_, in_=sv.to_broadcast((C, T)))
            pwarm = psw.tile([C, 4], F32, tag="warm")
            nc.tensor.matmul(pwarm, lhsT=wt, rhs=wt[:, 0:4], start=True, stop=True)

            for ld in range(NLOAD):
                xs = stage.tile([C, XT], F32R, tag="xs")
                nc.sync.dma_start(out=xs, in_=x[:, ld * XT:(ld + 1) * XT])
                os_ = ostage.tile([C, XT], F32, tag="os")
                for k in range(INNER):
                    pt = ps.tile([C, T], F32, tag="pt")
                    nc.tensor.matmul(pt, lhsT=wt, rhs=xs[:, k * T:(k + 1) * T],
                                     start=True, stop=True)
                    nc.vector.tensor_scalar(
                        out=os_[:, k * T:(k + 1) * T], in0=pt,
                        scalar1=0.0, scalar2=sv,
                        op0=mybir.AluOpType.max, op1=mybir.AluOpType.add)
                nc.scalar.dma_start(out=y[:, ld * XT:(ld + 1) * XT], in_=os_)
    nc.finalize()
    return nc


def _get(name):
    if name not in _cache:
        _cache[name] = {"k1": _build_k1, "k2": _build_k2, "k3": _build_k3}[name]()
    return _cache[name]


def _softmax64(v):
    v = v - v.max()
    e = np.exp(v)
    return e / e.sum()


def kernel(x, wqr, wvr, wup, wql, wvl, wsk1, wsk2, wres):
    x = np.asarray(x, dtype=np.float32)
    wup = np.asarray(wup, dtype=np.float64)
    wsk1 = np.asarray(wsk1, dtype=np.float64)
    wsk2 = np.asarray(wsk2, dtype=np.float64)
    wvl = np.asarray(wvl, dtype=np.float32)
    wres = np.asarray(wres, dtype=np.float32)

    xf = x.reshape(B, C, N)
    shards = []
    for k in range(NCORES):
        b, h = divmod(k, 2)
        shards.append(np.ascontiguousarray(xf[b, :, h * NS:(h + 1) * NS]))

    # ---- pass 1 ----
    # w columns: [wqr (1) | wvr (32) | wql (32)]
    wstack = np.concatenate([np.asarray(wqr, np.float32),
                             np.asarray(wvr, np.float32),
                             np.asarray(wql, np.float32),
                             np.zeros((1, C), np.float32)], axis=0).T.copy()
    r1 = run_bass_kernel_spmd(
        _get("k1"),
        in_maps=[{"x": shards[k], "w": wstack} for k in range(NCORES)],
        core_ids=list(range(NCORES)),
    ).results

    ctxU = np.zeros((B, CH))
    gsum = np.zeros((B, CH))
    Z = np.zeros(B)
    for k in range(NCORES):
        b = k // 2
        p = r1[k]["part"].astype(np.float64)
        ctxU[b] += p[0:CH, 0::2].sum(-1)
        gsum[b] += p[CH:64, 1::2].sum(-1)
        Z[b] += p[64, 0::2].sum(-1)

    ctx = ctxU / Z[:, None]                       # [B, CH]
    mu = ctx.mean(-1, keepdims=True)
    var = ctx.var(-1, keepdims=True)
    ln = (ctx - mu) / np.sqrt(var + 1e-5)
    sa = np.clip((ln @ wup.T + 3.0) / 6.0, 0.0, 1.0)   # [B, C]
    avgx = np.stack([_softmax64(gsum[b] / N) for b in range(B)])  # [B, CH]

    # ---- pass 2 ----
    wvlT = wvl.T.copy()
    av_bf = [avgx[b].astype(np.float32).reshape(1, CH) for b in range(B)]
    r2 = run_bass_kernel_spmd(
        _get("k2"),
        in_maps=[{"x": shards[k], "w": wvlT, "avgx": av_bf[k // 2]}
                 for k in range(NCORES)],
        core_ids=list(range(NCORES)),
    ).results

    sca = np.zeros(B)
    for k in range(NCORES):
        sca[k // 2] += float(r2[k]["part"].astype(np.float64).sum())

    mca = sca / N
    u = sa * mca[:, None] + 1.0 / N               # [B, C]
    a1 = np.maximum(u @ wsk1.T, 0.0)
    a2 = np.maximum(a1 @ wsk2.T, 0.0)
    a = np.stack([_softmax64(a2[b]) for b in range(B)])
    S = a * sa * sca[:, None] + 1.0 - a           # [B, C]

    # ---- pass 3 ----
    wresT = wres.T.copy()
    sv = [S[b].astype(np.float32).reshape(C, 1) for b in range(B)]
    r3 = run_bass_kernel_spmd(
        _get("k3"),
        in_maps=[{"x": shards[k], "w": wresT, "svec": sv[k // 2]}
                 for k in range(NCORES)],
        core_ids=list(range(NCORES)),
    ).results

    out = np.empty((B, C, N), dtype=np.float32)
    for k in range(NCORES):
        b, h = divmod(k, 2)
        out[b, :, h * NS:(h + 1) * NS] = r3[k]["y"]
    return out.reshape(B, C, D, H, W)
